# revision 35
# baseline (speedup 1.0000x reference)
"""Trainium2 Bass kernel for per-variable-MLP GNN message passing.

Model (reference):
    adj  = ones(D,D) - eye(D)                       # var t cannot see itself
    h0   = leaky_relu(einsum('tij,bj->bti', w0*adjmask, x) + b0)
    h1   = leaky_relu(einsum('tij,btj->bti', w1, h0) + b1)
    out  = einsum('tij,btj->bti', w2, h1) + b2      # (B, D, O)

Sharding: the variable axis t (128) is split across 8 cores (16 vars each);
each core sees the full batch. Within a core, variables are processed in
pairs: a pair's two (64 x K) weight matrices are stacked/block-diagonalized
to fill the 128-wide tensor-engine array; activations live transposed
(feature-on-partition, batch-on-free).

Throughput structure (v2 -- wide-tile rework; 108us -> ~95.5us):
- The PSUM->SBUF bias+leaky epilogues are the hard floor (fp32 PSUM reads
  run at 1 elem/lane/cycle and only ScalarE + VectorE have PSUM ports;
  GpSimd has none, DMA cannot touch PSUM). ScalarE runs fused
  Prelu-with-bias; VectorE runs a CUSTOM DVE op LEAKY_BIAS_ANT =
  max(z+b, alpha*(z+b)) registered at import time, so BOTH engines
  retire a tile in ONE instruction. Floor: 17 wide tiles/iteration
  ~= 9.6us/iter x 8 iters ~= 77us; both engines measure ~96% packed.
- 1024-col PSUM tiles spanning TWO banks: each epilogue instruction
  reads 1024 elems/lane, amortizing the ~175ns fixed ACT/DVE overhead
  (PSUM access latency + decode) and halving semaphore traffic on the
  two port-bound engines (ACT ~1060ns, DVE ~1210ns full-clock/tile).
  Single PSUM pool: 4 slots x [128,1024] = all 8 banks, shared by
  L0/L1/L2 in allocation order (slot WAR gives 4-alloc slack).
  MEASURED DEAD ENDS: 3 wide slots + dedicated z2 banks (+15us, the
  3-slot L0/L1 rotation stalls); interleaving the L2 block into A/B
  emission (+1..5us, early z2 alloc tightens every later slot WAR);
  chunk 0 as 2x512 sub-passes (confounded by a downclock era, retest
  if revisited -- SPLIT_CHUNK0 flag kept, off).
- Emission order per iteration k: A(L0 chunk k), B(L1 chunk k-1),
  C(L2 chunk k-2) -- C LAST is load-bearing: its first MM (q
  descending) reads the LAST-retired h1 tile, and the in-order PE
  queue stalls ~2us/iter if C is emitted before a full iteration of
  other work exists between B(k-2) and C(k-2).
- Each pair's two 512-col matmuls (same stationary weights) issue
  back-to-back at the 215ns streaming cadence; batch chunk = 1024.
- L2 packs ALL 8 pairs into one PSUM tile per 512-half at
  tile_position (0,32q) (M=8 zero-padded pB first then M=4 pA
  accumulate) -- adjacent col-group matmuls stream ~3ns apart. Last
  chunk drains q-ASCENDING with B-pairs emitted 4,5,6,7 first and the
  final evac split across both engines.
- Epilogues greedily time-balanced across ScalarE/VectorE (1069:1213
  weights); h tiles are tagged per engine (ha/hv) so slot WAW deps
  stay same-engine. The L2 bias-add is Prelu(alpha=1.0) == identity
  on ScalarE / tensor_scalar_add on VectorE.
- Head (measured): ~7.2us framework boot + HAM PE clock ramp (K=4
  half-clock for ~3.4us of CONTINUOUS PE activity -- idle gaps reset
  it, hence NWARM=7 warmup MMs bridging until the input DMAs land).
  Early DMAs round-robin queue-1 bandwidth across ALL queued
  transfers, so ONLY chunk-0/1 x halves + w0 prefetch up front
  (contiguous half-chunk-major layout); later chunks + w1 stream
  just-in-time one chunk ahead from inside the loop. Tiny bias loads
  trigger from the GpSimd queue (DMA triggers cost ~607ns each and
  serialize per queue). A dummy activation hoists the 1.3us
  ACT_TABLE_LOAD into the boot window.
- Tail: ~1us drain + ~9us framework teardown (fixed semaphore-reset
  sweep + cross-queue barriers; not kernel-controllable).
- Output: full (128, 1024) fp16 tiles per chunk (one DMA each; invalid
  rows are garbage the host drops) -- host reassembles (8192, 128, 2).

Matmuls run in fp16 (1 col/cycle on the PE, fp32 accumulate in PSUM).
fp8 was evaluated and rejected: e4m3 injects ~5% rms error per layer
vs the 2e-2 tolerance (fp16 measures 6.7e-4).

NOTE run-to-run variance: the part oscillates between sustained clock
states (~18% spread, ACTIVATE avg 1120ns fast vs 1330ns slow) --
normalize A/B timings by the ACTIVATE-duration clock proxy before
trusting a comparison.
"""

import numpy as np

import concourse.bass as bass
import concourse.mybir as mybir
import concourse.tile as tile
from concourse import bacc, bass_utils


def _register_leaky_bias():
    """Register a fused leaky_relu(z + bias) custom DVE op so VectorE can
    retire a PSUM epilogue tile in ONE instruction (the PSUM read is the
    port-bound cost; stock ops need add + stt = two passes).

    out = max(in0 + s0, (in0 + s0) * s1); s0 = per-partition bias AP,
    s1 = leaky slope immediate.
    """
    from concourse import dve_ops
    from concourse.dve_spec import Spec, Src0, C0, C1, maxx, lower, _has_src1
    from concourse.dve_table_gen import dve_ver_for
    from concourse.dve_uop import DveOpSpec

    name = "LEAKY_BIAS_ANT"
    if name in dve_ops._SUB_OPCODE_FOR_NAME:
        return next(op for op in dve_ops.OPS if op.name == name)

    v = Src0 + C0
    spec = Spec(
        body=maxx(v, v * C1),
        reference=lambda in0, in1, s0, s1, imm2: np.maximum(
            in0 + s0, (in0 + s0) * s1
        ),
    )
    row = 1 + len(dve_ops.OPS)
    shas = {}
    for ver in ("v3", "v4"):
        uops = lower(spec, ver=ver)
        shas[ver] = DveOpSpec(
            name=name, opcode=row, uops=uops, rd1_en=_has_src1(spec)
        ).sha(ver)
    op = dve_ops.DveOp(name, spec, subdim=False, uops_sha=shas)
    dve_ops.OPS.append(op)
    dve_ops.CUSTOM_DVE_SPECS[name] = spec
    dve_ops._SUB_OPCODE_FOR_NAME[name] = row
    return op


LEAKY_BIAS = _register_leaky_bias()

F32 = mybir.dt.float32
DT = mybir.dt.float16
NPDT = np.float16

B = 8192  # batch
D = 128  # num variables (t)
H = 64  # hidden
O = 2  # output dim per variable
NCORES = 8
TPC = D // NCORES  # vars per core = 16
NPAIR = TPC // 2  # 8
CH = 512  # matmul moving-dim max / psum bank (fp32)
CW = 1024  # wide batch chunk = 2 psum banks per tile
NCH = B // CW  # 8
ALPHA = 0.01  # leaky_relu slope
NWARM = 7  # PE warmup matmuls (HAM clock ramp over the DMA window)
SPLIT_CHUNK0 = False  # chunk 0 as two 512 sub-passes (early-DMA latency hiding)

Prelu = mybir.ActivationFunctionType.Prelu

# per-wide-tile epilogue costs (ns, cost model): used for greedy balance
COST_ACT = 1069.0
COST_ACT_N = 640.0  # 512-col variant
COST_DVE = 1213.0
COST_DVE_N = 745.0


def _build_program():
    nc = bacc.Bacc(trn_type="TRN2")

    # x half-chunk-major: xt[h] is a contiguous (128, CH) block in DRAM
    xt = nc.dram_tensor("xt", (2 * NCH, D, CH), DT, kind="ExternalInput")
    w0t = nc.dram_tensor("w0t", (D, NPAIR * 128), DT, kind="ExternalInput")
    w1bd = nc.dram_tensor("w1bd", (128, NPAIR * 128), DT, kind="ExternalInput")
    w2bd = nc.dram_tensor("w2bd", (128, NPAIR * 8), DT, kind="ExternalInput")
    b0c = nc.dram_tensor("b0c", (128, NPAIR), F32, kind="ExternalInput")
    b1c = nc.dram_tensor("b1c", (128, NPAIR), F32, kind="ExternalInput")
    # b2c: per-partition bias; partition 32q+r = pair q row r,
    # partition 32q+4+r = pair 4+q row r
    b2c = nc.dram_tensor("b2c", (128, 1), F32, kind="ExternalInput")
    # full-height output tiles, chunk-major; host drops the garbage rows
    otB = nc.dram_tensor("otB", (NCH, 128, CW), DT, kind="ExternalOutput")

    with tile.TileContext(nc) as tc:
        with (
            tc.tile_pool(name="wp", bufs=1) as wp,
            tc.tile_pool(name="hp", bufs=36) as hp,
            tc.tile_pool(name="op", bufs=3) as op,
            tc.tile_pool(name="zp", bufs=4, space="PSUM") as zp,
        ):
            xs = wp.tile([D, B], DT)
            w0s = wp.tile([D, NPAIR * 128], DT)
            w1s = wp.tile([128, NPAIR * 128], DT)
            w2s = wp.tile([128, NPAIR * 8], DT)
            b0s = wp.tile([128, NPAIR], F32)
            b1s = wp.tile([128, NPAIR], F32)
            b2s = wp.tile([128, 1], F32)
            # trigger order = need order: the first L0 matmul is gated on
            # x half 0 + w0 pair 0 only (w0 is split so pair 0 lands fast).
            # The tiny bias/w2 loads trigger from the otherwise-idle GpSimd
            # queue: DMA triggers cost ~607ns EACH and serialize per queue,
            # so on Sync alone the b0 bias (which gates the FIRST epilogue)
            # only landed at ~12us.
            # CRITICAL-ONLY prefetch: the input DMA queue round-robins
            # bandwidth across ALL queued transfers, so prefetching every
            # x chunk up front starved the chunk-0/1 loads the pipeline
            # start gates on (they completed at ~14-16us instead of ~10).
            # Later chunks + w1 stream just-in-time from inside the loop.
            nc.sync.dma_start(xs[:, 0:CH], xt[0])
            nc.sync.dma_start(w0s[:, 0:128], w0t[:, 0:128])
            nc.sync.dma_start(w0s[:, 128:], w0t[:, 128:])
            nc.sync.dma_start(xs[:, CH:CW], xt[1])
            nc.gpsimd.dma_start(b0s[:], b0c[:])
            nc.gpsimd.dma_start(b1s[:], b1c[:])
            nc.gpsimd.dma_start(w2s[:], w2bd[:])
            nc.gpsimd.dma_start(b2s[:], b2c[:])

            # greedy ScalarE/VectorE balance over accumulated epilogue ns
            eng_t = {"act": 0.0, "cdve": 0.0}

            def pick_engine(ca=COST_ACT, cd=COST_DVE):
                k = "act" if eng_t["act"] + ca <= eng_t["cdve"] + cd else "cdve"
                eng_t[k] += ca if k == "act" else cd
                return k

            def leaky_epilogue(z, bias_col, name):
                """fp16 SBUF tile = leaky_relu(z + bias), z in PSUM.

                h tiles are segregated per engine (tags ha/hv) so the
                slot WAW dep is same-engine and folds into queue order
                instead of spilling a standalone semaphore wait."""
                kind = pick_engine()
                if kind == "act":
                    dst = hp.tile([128, CW], DT, tag="ha", bufs=20,
                                  name=name)
                    nc.scalar.activation(
                        dst[:], z[:], Prelu, bias=bias_col, scale=1.0,
                        alpha=ALPHA,
                    )
                else:  # fused leaky_relu(z + bias) in one VectorE op
                    dst = hp.tile([128, CW], DT, tag="hv", bufs=18,
                                  name=name)
                    nc.vector._custom_dve(
                        LEAKY_BIAS, out=dst[:], in0=z[:], s0=bias_col,
                        s1=ALPHA,
                    )
                return dst

            # PE warmup: dummy matmuls with no input-DMA dependency so the
            # HAM clock-gate reaches 8/8 while the input DMAs run.
            # memset on Vector (idle until ~13.5us): GpSimd's queue is
            # occupied by the bias DMA triggers above, and the warmup
            # matmuls gate the PE clock ramp -- warm must land ASAP
            warm = wp.tile([128, CH], DT, name="warm")
            nc.vector.memset(warm[:], 0.0)
            # dummy activation: hoists the 1.3us ACT_TABLE_LOAD (emitted
            # lazily before the first ACTIVATE) into the DMA window -- it
            # has no input deps but was measured running at ~12us behind
            # the scheduler's hoisted DMA wait
            dumm = wp.tile([128, 1], DT, name="dumm")
            nc.scalar.activation(dumm[:], warm[:, 0:1], Prelu, scale=1.0,
                                 alpha=ALPHA)
            wps = zp.tile([128, CW], F32, name="warmps", tag="z")
            for _ in range(NWARM):
                nc.tensor.matmul(wps[:, 0:CH], warm[:, 0:128], warm[:],
                                 start=True, stop=True)

            # per-chunk state carried across pipeline stages
            h0_tiles = [None] * NCH
            h1_tiles = [None] * NCH

            def emit_A_pair(k, p):
                z0 = zp.tile([128, CW], F32, tag="z", name=f"z0_{k}_{p}")
                for half in range(2):
                    nc.tensor.matmul(
                        z0[:, half * CH : half * CH + CH],
                        w0s[:, bass.ts(p, 128)],
                        xs[:, k * CW + half * CH : k * CW + half * CH + CH],
                        start=True, stop=True,
                    )
                return leaky_epilogue(z0, b0s[:, p : p + 1], f"h0_{k}_{p}")

            def emit_A_chunk0():
                """Chunk 0 runs as two 512-wide sub-passes: the early input
                DMAs have ~2-3us latency, so the first wide epilogue would
                otherwise idle both port engines until x half 1 lands
                (~12us); half-0-only work starts ~2.5us sooner."""
                tiles = []
                for p in range(NPAIR):
                    tag = "ha" if p % 2 == 0 else "hv"
                    bufs = 20 if tag == "ha" else 18
                    tiles.append(hp.tile([128, CW], DT, tag=tag, bufs=bufs,
                                         name=f"h0_0_{p}"))
                for half in range(2):
                    for p in range(NPAIR):
                        z0 = zp.tile([128, CW], F32, tag="z",
                                     name=f"z0_0_{p}_{half}")
                        nc.tensor.matmul(
                            z0[:, 0:CH], w0s[:, bass.ts(p, 128)],
                            xs[:, half * CH : half * CH + CH],
                            start=True, stop=True,
                        )
                        dst = tiles[p][:, half * CH : half * CH + CH]
                        if pick_engine() == "act":
                            nc.scalar.activation(
                                dst, z0[:, 0:CH], Prelu,
                                bias=b0s[:, p : p + 1], scale=1.0,
                                alpha=ALPHA,
                            )
                        else:
                            nc.vector._custom_dve(
                                LEAKY_BIAS, out=dst, in0=z0[:, 0:CH],
                                s0=b0s[:, p : p + 1], s1=ALPHA,
                            )
                return tiles

            def emit_B_pair(c, p):
                z1 = zp.tile([128, CW], F32, tag="z", name=f"z1_{c}_{p}")
                t0 = h0_tiles[c][p]
                for half in range(2):
                    nc.tensor.matmul(
                        z1[:, half * CH : half * CH + CH],
                        w1s[:, bass.ts(p, 128)],
                        t0[:, half * CH : half * CH + CH],
                        start=True, stop=True,
                    )
                return leaky_epilogue(z1, b1s[:, p : p + 1], f"h1_{c}_{p}")

            def emit_C_half(c, z2, half, qorder):
                hs = slice(half * CH, half * CH + CH)
                for q in qorder:
                    # pair 4+q: M=8 zero-padded weights write zeros into
                    # rows 32q..+4, data into rows 32q+4..+8
                    pB = 4 + q
                    nc.tensor.matmul(
                        z2[32 * q : 32 * q + 8, hs],
                        w2s[:, 8 * pB : 8 * pB + 8],
                        h1_tiles[c][pB][:, hs],
                        start=True, stop=False,
                        tile_position=(0, 32 * q),
                    )
                for q in qorder:
                    # pair q accumulates into rows 32q..32q+4
                    nc.tensor.matmul(
                        z2[32 * q : 32 * q + 4, hs],
                        w2s[:, 8 * q : 8 * q + 4],
                        h1_tiles[c][q][:, hs],
                        start=False, stop=True,
                        tile_position=(0, 32 * q),
                    )

            def emit_C_evac(c, z2):
                # the evac is ALWAYS split into two 512 halves, one per
                # engine pick: a single wide evac head-of-line blocks its
                # engine's FIFO queue for the full 16-MM L2 block latency
                # (~450ns idle once per iteration, measured); halves also
                # retire the z2 slot sooner for the next iteration's WAR
                ob = op.tile([128, CW], DT, tag="oba")
                if c == NCH - 1:
                    kinds = ("act", "cdve")  # drain: force one per engine
                else:
                    kinds = (pick_engine(COST_ACT_N, COST_DVE_N),
                             pick_engine(COST_ACT_N, COST_DVE_N))
                for half, kind in enumerate(kinds):
                    hs = slice(half * CH, half * CH + CH)
                    if kind == "act":
                        # Prelu with alpha=1 is identity: bias-add on ACT
                        nc.scalar.activation(
                            ob[:, hs], z2[:, hs], Prelu, bias=b2s[:, 0:1],
                            scale=1.0, alpha=1.0,
                        )
                    else:
                        nc.vector.tensor_scalar_add(
                            ob[:, hs], z2[:, hs], b2s[:, 0:1]
                        )
                if c == NCH - 1:
                    # the end-of-kernel barrier waits on the final DMA
                    # transfer: ship only the 104 rows that hold valid
                    # output, one DMA per half right behind its evac
                    nc.sync.dma_start(otB[c][0:104, 0:CH], ob[0:104, 0:CH])
                    nc.sync.dma_start(otB[c][0:104, CH:CW], ob[0:104, CH:CW])
                else:
                    nc.sync.dma_start(otB[c], ob[:])
                h1_tiles[c] = None

            for k in range(NCH + 2):
                cA = k if k < NCH else None
                cB = k - 1 if 1 <= k <= NCH else None
                cC = k - 2 if k >= 2 else None
                # last chunk: emit the pB pairs (4-7) first and run the L2
                # drain q-ASCENDING so its first matmuls read the
                # FIRST-retired h1 tiles and overlap the tail epilogues
                qorder = range(4) if cC == NCH - 1 else range(3, -1, -1)
                pordB = (
                    [4, 5, 6, 7, 0, 1, 2, 3] if cB == NCH - 1
                    else list(range(NPAIR))
                )

                # main-stage emission units to interleave the L2 rounds
                # with: the 16-MM L2 block alone occupies ~1us of the
                # in-order PE queue while producing one epilogue tile, so
                # emitted standalone it runs both port engines dry at every
                # iteration boundary (~0.5us each, measured)
                units = []
                if cA is not None:
                    units += [("A", cA, p) for p in range(NPAIR)]
                if cB is not None:
                    units += [("B", cB, p) for p in pordB]

                outA = {}
                outB = {}
                # NOTE: interleaving the L2 block into the A/B emission was
                # measured SLOWER (+2..5us): the early z2 slot allocation
                # tightens the 4-slot WAR window for every subsequent tile.
                # Emitting C after all of A and B (z2 = last alloc of the
                # iteration) is the fastest measured arrangement.
                # just-in-time input streaming, one chunk ahead (chunk
                # k+1 loads during iteration k, consumed in iteration k+1)
                if k == 0:
                    nc.sync.dma_start(w1s[:], w1bd[:])
                if k + 1 < NCH:
                    for hh in (2 * (k + 1), 2 * (k + 1) + 1):
                        nc.sync.dma_start(
                            xs[:, hh * CH : (hh + 1) * CH], xt[hh]
                        )

                if k == 0 and SPLIT_CHUNK0:
                    h0_tiles[0] = emit_A_chunk0()
                    units = [u for u in units if u[0] != "A"]
                for st, c, p in units:
                    if st == "A":
                        outA[p] = emit_A_pair(c, p)
                    else:
                        outB[p] = emit_B_pair(c, p)
                if cC is not None:
                    z2 = zp.tile([128, CW], F32, name=f"z2_{cC}", tag="z")
                    emit_C_half(cC, z2, 0, qorder)
                    emit_C_half(cC, z2, 1, qorder)
                    emit_C_evac(cC, z2)
                if cA is not None and not (cA == 0 and SPLIT_CHUNK0):
                    h0_tiles[cA] = [outA[p] for p in range(NPAIR)]
                if cB is not None:
                    h1_tiles[cB] = [outB[p] for p in range(NPAIR)]
                    h0_tiles[cB] = None

    nc.finalize()
    return nc


_prog = None


def _get_program():
    global _prog
    if _prog is None:
        _prog = _build_program()
    return _prog


def _shard_inputs(x, w0, w1, w2, b0, b1, b2):
    """Host-side relayout + t-sharding. Returns list of 8 in_maps."""
    x = np.asarray(x, np.float32)
    w0 = np.array(w0, np.float32)  # copy: we zero the adjacency diagonal
    w1 = np.asarray(w1, np.float32)
    w2 = np.asarray(w2, np.float32)
    b0 = np.asarray(b0, np.float32)
    b1 = np.asarray(b1, np.float32)
    b2 = np.asarray(b2, np.float32)

    # adjacency mask: variable t cannot see itself -> w0[t, :, t] = 0
    ar = np.arange(D)
    w0[ar, :, ar] = 0.0

    # half-chunk-major x: (2*NCH, 128, CH), each half contiguous in DRAM
    xt = np.ascontiguousarray(
        x.T.reshape(D, 2 * NCH, CH).transpose(1, 0, 2)
    ).astype(NPDT)

    in_maps = []
    for c in range(NCORES):
        ts_ = slice(c * TPC, (c + 1) * TPC)
        w0c, w1c, w2c = w0[ts_], w1[ts_], w2[ts_]
        b0cc, b1cc, b2cc = b0[ts_], b1[ts_], b2[ts_]

        # w0t: (128 j, pair*128 + [ta's 64 i | tb's 64 i])
        w0T = w0c.transpose(0, 2, 1)  # (16, 128 j, 64 i)
        w0t_ = np.ascontiguousarray(
            w0T.reshape(NPAIR, 2, D, H).transpose(2, 0, 1, 3).reshape(D, NPAIR * 128)
        ).astype(NPDT)

        # w1bd: per-pair 128x128 block-diagonal; K rows = h0 pair stack.
        bd1 = np.zeros((NPAIR, 128, 128), np.float32)
        for p in range(NPAIR):
            te, to = w1c[2 * p].T, w1c[2 * p + 1].T  # (in, out) each (64,64)
            bd1[p, 0:H, 0:H] = te
            bd1[p, H:128, H:128] = to
        w1bd_ = np.ascontiguousarray(
            bd1.transpose(1, 0, 2).reshape(128, NPAIR * 128)
        ).astype(NPDT)

        b1c_ = np.ascontiguousarray(
            b1cc.reshape(NPAIR, 128).T
        ).astype(np.float32)

        # w2bd: (128 K, pair*8 + [4 zero-pad | ta o0, ta o1, tb o0, tb o1]);
        # B pairs sit 4 cols right (zero-padded M=8 write clears the rows
        # the A pairs later accumulate into)
        bd2 = np.zeros((NPAIR, 128, 8), np.float32)
        for p in range(NPAIR):
            te, to = w2c[2 * p].T, w2c[2 * p + 1].T  # (64, 2) each
            off = 0 if p < 4 else 4
            bd2[p, 0:H, off : off + 2] = te
            bd2[p, H:128, off + 2 : off + 4] = to
        w2bd_ = np.ascontiguousarray(
            bd2.transpose(1, 0, 2).reshape(128, NPAIR * 8)
        ).astype(NPDT)

        b0c_ = np.ascontiguousarray(b0cc.reshape(NPAIR, 128).T).astype(np.float32)
        # b2 bias layout for col-packed L2 psums: partition 32q+r = pair q
        # row r; partition 32q+4+r = pair 4+q row r (r = 2*two + o)
        b2q = b2cc.reshape(NPAIR, 4)
        b2c_ = np.zeros((128, 1), np.float32)
        for q in range(4):
            b2c_[32 * q : 32 * q + 4, 0] = b2q[q]
            b2c_[32 * q + 4 : 32 * q + 8, 0] = b2q[4 + q]

        in_maps.append(
            {
                "xt": xt,
                "w0t": w0t_,
                "w1bd": w1bd_,
                "w2bd": w2bd_,
                "b0c": b0c_,
                "b1c": b1c_,
                "b2c": b2c_,
            }
        )
    return in_maps


# full-height output rows: partition 32q+r = pair q row r,
# partition 32q+4+r = pair 4+q row r (r = 2*two + o)
_SEL = np.array(
    [
        32 * p + r if p < 4 else 32 * (p - 4) + 4 + r
        for p in range(NPAIR)
        for r in range(4)
    ]
)


def _unshard_outputs(results):
    out = np.empty((B, D, O), np.float32)
    for c in range(NCORES):
        ot = results[c]["otB"]  # (NCH, 128, CW) fp16
        ott = ot.transpose(1, 0, 2).reshape(128, B)[_SEL].astype(np.float32)
        blk = ott.reshape(NPAIR, 2, O, B).transpose(3, 0, 1, 2).reshape(B, TPC, O)
        out[:, c * TPC : (c + 1) * TPC, :] = blk
    return out


def kernel(x, w0, w1, w2, b0, b1, b2):
    nc = _get_program()
    in_maps = _shard_inputs(x, w0, w1, w2, b0, b1, b2)
    res = bass_utils.run_bass_kernel_spmd(nc, in_maps, core_ids=list(range(NCORES)))
    return _unshard_outputs(res.results)


# revision 36
# speedup vs baseline: 1.0094x; 1.0094x over previous
"""Trainium2 Bass kernel for per-variable-MLP GNN message passing.

Model (reference):
    adj  = ones(D,D) - eye(D)                       # var t cannot see itself
    h0   = leaky_relu(einsum('tij,bj->bti', w0*adjmask, x) + b0)
    h1   = leaky_relu(einsum('tij,btj->bti', w1, h0) + b1)
    out  = einsum('tij,btj->bti', w2, h1) + b2      # (B, D, O)

Sharding: the variable axis t (128) is split across 8 cores (16 vars each);
each core sees the full batch. Within a core, variables are processed in
pairs: a pair's two (64 x K) weight matrices are stacked/block-diagonalized
to fill the 128-wide tensor-engine array; activations live transposed
(feature-on-partition, batch-on-free).

Throughput structure (v2 -- wide-tile rework of the session baseline):
- The PSUM->SBUF bias+leaky epilogues are the hard floor (fp32 PSUM reads
  run at 1 elem/lane/cycle and only ScalarE + VectorE have PSUM ports).
  ScalarE runs fused Prelu-with-bias; VectorE runs a CUSTOM DVE op
  LEAKY_BIAS_ANT = max(z+b, alpha*(z+b)) registered at import time, so
  BOTH engines retire a tile in ONE instruction.
- v2 uses 1024-col PSUM tiles spanning TWO banks: each epilogue
  instruction reads 1024 elems/lane, amortizing the ~175ns fixed
  ACT/DVE instruction overhead (PSUM access latency + decode) and
  halving the semaphore traffic on the two port-bound engines.
  Single PSUM pool: 4 slots x [128,1024] = all 8 banks, shared by
  L0/L1/L2 in allocation order (slot WAR gives 4-alloc slack).
- Each pair's two 512-col matmuls (same stationary weights) issue
  back-to-back at the 215ns streaming cadence; batch chunk = 1024.
- L2 packs ALL 8 pairs into one PSUM tile per 512-half at
  tile_position (0,32q) (q descending, M=8 zero-padded pB first then
  M=4 pA accumulate) -- adjacent col-group matmuls stream concurrently.
- Epilogues are greedily load-balanced across ScalarE (1028ns/wide
  tile) and VectorE (1237ns/wide tile); the L2 bias-add is a
  Prelu(alpha=1) on ScalarE / tensor_scalar_add on VectorE.
- x is staged chunk-major in DRAM so each chunk DMA is contiguous
  (the old column-sliced loads read 1KB strided rows at ~39GB/s and
  delayed pipeline start by ~3us).
- Output: full (128, 1024) fp16 tiles per chunk (one DMA each; invalid
  rows are garbage the host drops) -- host reassembles (8192, 128, 2).

Matmuls run in fp16 (1 col/cycle on the PE, fp32 accumulate in PSUM).
"""

import numpy as np

import concourse.bass as bass
import concourse.mybir as mybir
import concourse.tile as tile
from concourse import bacc, bass_utils


def _register_leaky_bias():
    """Register a fused leaky_relu(z + bias) custom DVE op so VectorE can
    retire a PSUM epilogue tile in ONE instruction (the PSUM read is the
    port-bound cost; stock ops need add + stt = two passes).

    out = max(in0 + s0, (in0 + s0) * s1); s0 = per-partition bias AP,
    s1 = leaky slope immediate.
    """
    from concourse import dve_ops
    from concourse.dve_spec import Spec, Src0, C0, C1, maxx, lower, _has_src1
    from concourse.dve_table_gen import dve_ver_for
    from concourse.dve_uop import DveOpSpec

    name = "LEAKY_BIAS_ANT"
    if name in dve_ops._SUB_OPCODE_FOR_NAME:
        return next(op for op in dve_ops.OPS if op.name == name)

    v = Src0 + C0
    spec = Spec(
        body=maxx(v, v * C1),
        reference=lambda in0, in1, s0, s1, imm2: np.maximum(
            in0 + s0, (in0 + s0) * s1
        ),
    )
    row = 1 + len(dve_ops.OPS)
    shas = {}
    for ver in ("v3", "v4"):
        uops = lower(spec, ver=ver)
        shas[ver] = DveOpSpec(
            name=name, opcode=row, uops=uops, rd1_en=_has_src1(spec)
        ).sha(ver)
    op = dve_ops.DveOp(name, spec, subdim=False, uops_sha=shas)
    dve_ops.OPS.append(op)
    dve_ops.CUSTOM_DVE_SPECS[name] = spec
    dve_ops._SUB_OPCODE_FOR_NAME[name] = row
    return op


LEAKY_BIAS = _register_leaky_bias()

F32 = mybir.dt.float32
DT = mybir.dt.float16
NPDT = np.float16

B = 8192  # batch
D = 128  # num variables (t)
H = 64  # hidden
O = 2  # output dim per variable
NCORES = 8
TPC = D // NCORES  # vars per core = 16
NPAIR = TPC // 2  # 8
CH = 512  # matmul moving-dim max / psum bank (fp32)
CW = 1024  # wide batch chunk = 2 psum banks per tile
NCH = B // CW  # 8
ALPHA = 0.01  # leaky_relu slope
NWARM = 7  # PE warmup matmuls (HAM clock ramp over the DMA window)
SPLIT_CHUNK0 = False  # chunk 0 as two 512 sub-passes (early-DMA latency hiding)

Prelu = mybir.ActivationFunctionType.Prelu

# per-wide-tile epilogue costs (ns, cost model): used for greedy balance
COST_ACT = 1069.0
COST_DVE = 1213.0


def _build_program():
    nc = bacc.Bacc(trn_type="TRN2")

    # x half-chunk-major: xt[h] is a contiguous (128, CH) block in DRAM
    xt = nc.dram_tensor("xt", (2 * NCH, D, CH), DT, kind="ExternalInput")
    w0t = nc.dram_tensor("w0t", (D, NPAIR * 128), DT, kind="ExternalInput")
    w1bd = nc.dram_tensor("w1bd", (128, NPAIR * 128), DT, kind="ExternalInput")
    w2bd = nc.dram_tensor("w2bd", (128, NPAIR * 8), DT, kind="ExternalInput")
    b0c = nc.dram_tensor("b0c", (128, NPAIR), F32, kind="ExternalInput")
    b1c = nc.dram_tensor("b1c", (128, NPAIR), F32, kind="ExternalInput")
    # b2c: per-partition bias; partition 32q+r = pair q row r,
    # partition 32q+4+r = pair 4+q row r
    b2c = nc.dram_tensor("b2c", (128, 1), F32, kind="ExternalInput")
    # full-height output tiles, chunk-major; host drops the garbage rows
    otB = nc.dram_tensor("otB", (NCH, 128, CW), DT, kind="ExternalOutput")

    with tile.TileContext(nc) as tc:
        with (
            tc.tile_pool(name="wp", bufs=1) as wp,
            tc.tile_pool(name="hp", bufs=36) as hp,
            tc.tile_pool(name="op", bufs=3) as op,
            tc.tile_pool(name="zp", bufs=4, space="PSUM") as zp,
        ):
            xs = wp.tile([D, B], DT)
            w0s = wp.tile([D, NPAIR * 128], DT)
            w1s = wp.tile([128, NPAIR * 128], DT)
            w2s = wp.tile([128, NPAIR * 8], DT)
            b0s = wp.tile([128, NPAIR], F32)
            b1s = wp.tile([128, NPAIR], F32)
            b2s = wp.tile([128, 1], F32)
            # trigger order = need order: the first L0 matmul is gated on
            # x half 0 + w0 pair 0 only (w0 is split so pair 0 lands fast).
            # The tiny bias/w2 loads trigger from the otherwise-idle GpSimd
            # queue: DMA triggers cost ~607ns EACH and serialize per queue,
            # so on Sync alone the b0 bias (which gates the FIRST epilogue)
            # only landed at ~12us.
            # CRITICAL-ONLY prefetch: the input DMA queue round-robins
            # bandwidth across ALL queued transfers, so prefetching every
            # x chunk up front starved the chunk-0/1 loads the pipeline
            # start gates on (they completed at ~14-16us instead of ~10).
            # Later chunks + w1 stream just-in-time from inside the loop.
            nc.sync.dma_start(xs[:, 0:CH], xt[0])
            nc.sync.dma_start(w0s[:, 0:128], w0t[:, 0:128])
            nc.sync.dma_start(w0s[:, 128:], w0t[:, 128:])
            nc.sync.dma_start(xs[:, CH:CW], xt[1])
            nc.gpsimd.dma_start(b0s[:], b0c[:])
            nc.gpsimd.dma_start(b1s[:], b1c[:])
            nc.gpsimd.dma_start(w2s[:], w2bd[:])
            nc.gpsimd.dma_start(b2s[:], b2c[:])

            # greedy ScalarE/VectorE balance over accumulated epilogue ns
            eng_t = {"act": 0.0, "cdve": 0.0}

            def pick_engine():
                k = "act" if eng_t["act"] + COST_ACT <= eng_t["cdve"] + COST_DVE else "cdve"
                eng_t[k] += COST_ACT if k == "act" else COST_DVE
                return k

            def leaky_epilogue(z, bias_col, name):
                """fp16 SBUF tile = leaky_relu(z + bias), z in PSUM.

                h tiles are segregated per engine (tags ha/hv) so the
                slot WAW dep is same-engine and folds into queue order
                instead of spilling a standalone semaphore wait."""
                kind = pick_engine()
                if kind == "act":
                    dst = hp.tile([128, CW], DT, tag="ha", bufs=20,
                                  name=name)
                    nc.scalar.activation(
                        dst[:], z[:], Prelu, bias=bias_col, scale=1.0,
                        alpha=ALPHA,
                    )
                else:  # fused leaky_relu(z + bias) in one VectorE op
                    dst = hp.tile([128, CW], DT, tag="hv", bufs=18,
                                  name=name)
                    nc.vector._custom_dve(
                        LEAKY_BIAS, out=dst[:], in0=z[:], s0=bias_col,
                        s1=ALPHA,
                    )
                return dst

            # PE warmup: dummy matmuls with no input-DMA dependency so the
            # HAM clock-gate reaches 8/8 while the input DMAs run.
            # memset on Vector (idle until ~13.5us): GpSimd's queue is
            # occupied by the bias DMA triggers above, and the warmup
            # matmuls gate the PE clock ramp -- warm must land ASAP
            warm = wp.tile([128, CH], DT, name="warm")
            nc.vector.memset(warm[:], 0.0)
            # dummy activation: hoists the 1.3us ACT_TABLE_LOAD (emitted
            # lazily before the first ACTIVATE) into the DMA window -- it
            # has no input deps but was measured running at ~12us behind
            # the scheduler's hoisted DMA wait
            dumm = wp.tile([128, 1], DT, name="dumm")
            nc.scalar.activation(dumm[:], warm[:, 0:1], Prelu, scale=1.0,
                                 alpha=ALPHA)
            wps = zp.tile([128, CW], F32, name="warmps", tag="z")
            for _ in range(NWARM):
                nc.tensor.matmul(wps[:, 0:CH], warm[:, 0:128], warm[:],
                                 start=True, stop=True)

            # per-chunk state carried across pipeline stages
            h0_tiles = [None] * NCH
            h1_tiles = [None] * NCH

            def emit_A_pair(k, p):
                z0 = zp.tile([128, CW], F32, tag="z", name=f"z0_{k}_{p}")
                for half in range(2):
                    nc.tensor.matmul(
                        z0[:, half * CH : half * CH + CH],
                        w0s[:, bass.ts(p, 128)],
                        xs[:, k * CW + half * CH : k * CW + half * CH + CH],
                        start=True, stop=True,
                    )
                return leaky_epilogue(z0, b0s[:, p : p + 1], f"h0_{k}_{p}")

            def emit_A_chunk0():
                """Chunk 0 runs as two 512-wide sub-passes: the early input
                DMAs have ~2-3us latency, so the first wide epilogue would
                otherwise idle both port engines until x half 1 lands
                (~12us); half-0-only work starts ~2.5us sooner."""
                tiles = []
                for p in range(NPAIR):
                    tag = "ha" if p % 2 == 0 else "hv"
                    bufs = 20 if tag == "ha" else 18
                    tiles.append(hp.tile([128, CW], DT, tag=tag, bufs=bufs,
                                         name=f"h0_0_{p}"))
                for half in range(2):
                    for p in range(NPAIR):
                        z0 = zp.tile([128, CW], F32, tag="z",
                                     name=f"z0_0_{p}_{half}")
                        nc.tensor.matmul(
                            z0[:, 0:CH], w0s[:, bass.ts(p, 128)],
                            xs[:, half * CH : half * CH + CH],
                            start=True, stop=True,
                        )
                        dst = tiles[p][:, half * CH : half * CH + CH]
                        if pick_engine() == "act":
                            nc.scalar.activation(
                                dst, z0[:, 0:CH], Prelu,
                                bias=b0s[:, p : p + 1], scale=1.0,
                                alpha=ALPHA,
                            )
                        else:
                            nc.vector._custom_dve(
                                LEAKY_BIAS, out=dst, in0=z0[:, 0:CH],
                                s0=b0s[:, p : p + 1], s1=ALPHA,
                            )
                return tiles

            def emit_B_pair(c, p):
                z1 = zp.tile([128, CW], F32, tag="z", name=f"z1_{c}_{p}")
                t0 = h0_tiles[c][p]
                for half in range(2):
                    nc.tensor.matmul(
                        z1[:, half * CH : half * CH + CH],
                        w1s[:, bass.ts(p, 128)],
                        t0[:, half * CH : half * CH + CH],
                        start=True, stop=True,
                    )
                return leaky_epilogue(z1, b1s[:, p : p + 1], f"h1_{c}_{p}")

            def emit_C_half(c, z2, half, qorder):
                hs = slice(half * CH, half * CH + CH)
                for q in qorder:
                    # pair 4+q: M=8 zero-padded weights write zeros into
                    # rows 32q..+4, data into rows 32q+4..+8
                    pB = 4 + q
                    nc.tensor.matmul(
                        z2[32 * q : 32 * q + 8, hs],
                        w2s[:, 8 * pB : 8 * pB + 8],
                        h1_tiles[c][pB][:, hs],
                        start=True, stop=False,
                        tile_position=(0, 32 * q),
                    )
                for q in qorder:
                    # pair q accumulates into rows 32q..32q+4
                    nc.tensor.matmul(
                        z2[32 * q : 32 * q + 4, hs],
                        w2s[:, 8 * q : 8 * q + 4],
                        h1_tiles[c][q][:, hs],
                        start=False, stop=True,
                        tile_position=(0, 32 * q),
                    )

            def emit_C_evac(c, z2):
                if c == NCH - 1:
                    # drain fast: one 512 half per engine, two DMAs
                    ob = op.tile([128, CW], DT, tag="oba")
                    nc.scalar.activation(
                        ob[:, 0:CH], z2[:, 0:CH], Prelu,
                        bias=b2s[:, 0:1], scale=1.0, alpha=1.0,
                    )
                    nc.vector.tensor_scalar_add(
                        ob[:, CH:CW], z2[:, CH:CW], b2s[:, 0:1]
                    )
                    # ship only the 104 valid rows; halves go out on two
                    # DIFFERENT trigger queues so the final transfers (which
                    # the end-of-kernel barrier waits on) run in parallel
                    nc.sync.dma_start(otB[c][0:104, 0:CH], ob[0:104, 0:CH])
                    nc.gpsimd.dma_start(
                        otB[c][0:104, CH:CW], ob[0:104, CH:CW]
                    )
                elif pick_engine() == "act":
                    ob = op.tile([128, CW], DT, tag="oba")
                    # Prelu with alpha=1 is identity: bias-add on ACT
                    nc.scalar.activation(
                        ob[:], z2[:], Prelu, bias=b2s[:, 0:1],
                        scale=1.0, alpha=1.0,
                    )
                    nc.sync.dma_start(otB[c], ob[:])
                else:
                    ob = op.tile([128, CW], DT, tag="obv")
                    nc.vector.tensor_scalar_add(ob[:], z2[:], b2s[:, 0:1])
                    nc.sync.dma_start(otB[c], ob[:])
                h1_tiles[c] = None

            for k in range(NCH + 2):
                cA = k if k < NCH else None
                cB = k - 1 if 1 <= k <= NCH else None
                cC = k - 2 if k >= 2 else None
                # last chunk: emit the pB pairs (4-7) first and run the L2
                # drain q-ASCENDING so its first matmuls read the
                # FIRST-retired h1 tiles and overlap the tail epilogues
                qorder = range(4) if cC == NCH - 1 else range(3, -1, -1)
                pordB = (
                    [4, 5, 6, 7, 0, 1, 2, 3] if cB == NCH - 1
                    else list(range(NPAIR))
                )

                # main-stage emission units to interleave the L2 rounds
                # with: the 16-MM L2 block alone occupies ~1us of the
                # in-order PE queue while producing one epilogue tile, so
                # emitted standalone it runs both port engines dry at every
                # iteration boundary (~0.5us each, measured)
                units = []
                if cA is not None:
                    units += [("A", cA, p) for p in range(NPAIR)]
                if cB is not None:
                    units += [("B", cB, p) for p in pordB]

                outA = {}
                outB = {}
                # NOTE: interleaving the L2 block into the A/B emission was
                # measured SLOWER (+2..5us): the early z2 slot allocation
                # tightens the 4-slot WAR window for every subsequent tile.
                # Emitting C after all of A and B (z2 = last alloc of the
                # iteration) is the fastest measured arrangement.
                # just-in-time input streaming, one chunk ahead (chunk
                # k+1 loads during iteration k, consumed in iteration k+1)
                if k == 0:
                    nc.sync.dma_start(w1s[:], w1bd[:])
                if k + 1 < NCH:
                    for hh in (2 * (k + 1), 2 * (k + 1) + 1):
                        nc.sync.dma_start(
                            xs[:, hh * CH : (hh + 1) * CH], xt[hh]
                        )

                if k == 0 and SPLIT_CHUNK0:
                    h0_tiles[0] = emit_A_chunk0()
                    units = [u for u in units if u[0] != "A"]
                for st, c, p in units:
                    if st == "A":
                        outA[p] = emit_A_pair(c, p)
                    else:
                        outB[p] = emit_B_pair(c, p)
                if cC is not None:
                    z2 = zp.tile([128, CW], F32, name=f"z2_{cC}", tag="z")
                    emit_C_half(cC, z2, 0, qorder)
                    emit_C_half(cC, z2, 1, qorder)
                    emit_C_evac(cC, z2)
                if cA is not None and not (cA == 0 and SPLIT_CHUNK0):
                    h0_tiles[cA] = [outA[p] for p in range(NPAIR)]
                if cB is not None:
                    h1_tiles[cB] = [outB[p] for p in range(NPAIR)]
                    h0_tiles[cB] = None

    nc.finalize()
    return nc


_prog = None


def _get_program():
    global _prog
    if _prog is None:
        _prog = _build_program()
    return _prog


def _shard_inputs(x, w0, w1, w2, b0, b1, b2):
    """Host-side relayout + t-sharding. Returns list of 8 in_maps."""
    x = np.asarray(x, np.float32)
    w0 = np.array(w0, np.float32)  # copy: we zero the adjacency diagonal
    w1 = np.asarray(w1, np.float32)
    w2 = np.asarray(w2, np.float32)
    b0 = np.asarray(b0, np.float32)
    b1 = np.asarray(b1, np.float32)
    b2 = np.asarray(b2, np.float32)

    # adjacency mask: variable t cannot see itself -> w0[t, :, t] = 0
    ar = np.arange(D)
    w0[ar, :, ar] = 0.0

    # half-chunk-major x: (2*NCH, 128, CH), each half contiguous in DRAM
    xt = np.ascontiguousarray(
        x.T.reshape(D, 2 * NCH, CH).transpose(1, 0, 2)
    ).astype(NPDT)

    in_maps = []
    for c in range(NCORES):
        ts_ = slice(c * TPC, (c + 1) * TPC)
        w0c, w1c, w2c = w0[ts_], w1[ts_], w2[ts_]
        b0cc, b1cc, b2cc = b0[ts_], b1[ts_], b2[ts_]

        # w0t: (128 j, pair*128 + [ta's 64 i | tb's 64 i])
        w0T = w0c.transpose(0, 2, 1)  # (16, 128 j, 64 i)
        w0t_ = np.ascontiguousarray(
            w0T.reshape(NPAIR, 2, D, H).transpose(2, 0, 1, 3).reshape(D, NPAIR * 128)
        ).astype(NPDT)

        # w1bd: per-pair 128x128 block-diagonal; K rows = h0 pair stack.
        bd1 = np.zeros((NPAIR, 128, 128), np.float32)
        for p in range(NPAIR):
            te, to = w1c[2 * p].T, w1c[2 * p + 1].T  # (in, out) each (64,64)
            bd1[p, 0:H, 0:H] = te
            bd1[p, H:128, H:128] = to
        w1bd_ = np.ascontiguousarray(
            bd1.transpose(1, 0, 2).reshape(128, NPAIR * 128)
        ).astype(NPDT)

        b1c_ = np.ascontiguousarray(
            b1cc.reshape(NPAIR, 128).T
        ).astype(np.float32)

        # w2bd: (128 K, pair*8 + [4 zero-pad | ta o0, ta o1, tb o0, tb o1]);
        # B pairs sit 4 cols right (zero-padded M=8 write clears the rows
        # the A pairs later accumulate into)
        bd2 = np.zeros((NPAIR, 128, 8), np.float32)
        for p in range(NPAIR):
            te, to = w2c[2 * p].T, w2c[2 * p + 1].T  # (64, 2) each
            off = 0 if p < 4 else 4
            bd2[p, 0:H, off : off + 2] = te
            bd2[p, H:128, off + 2 : off + 4] = to
        w2bd_ = np.ascontiguousarray(
            bd2.transpose(1, 0, 2).reshape(128, NPAIR * 8)
        ).astype(NPDT)

        b0c_ = np.ascontiguousarray(b0cc.reshape(NPAIR, 128).T).astype(np.float32)
        # b2 bias layout for col-packed L2 psums: partition 32q+r = pair q
        # row r; partition 32q+4+r = pair 4+q row r (r = 2*two + o)
        b2q = b2cc.reshape(NPAIR, 4)
        b2c_ = np.zeros((128, 1), np.float32)
        for q in range(4):
            b2c_[32 * q : 32 * q + 4, 0] = b2q[q]
            b2c_[32 * q + 4 : 32 * q + 8, 0] = b2q[4 + q]

        in_maps.append(
            {
                "xt": xt,
                "w0t": w0t_,
                "w1bd": w1bd_,
                "w2bd": w2bd_,
                "b0c": b0c_,
                "b1c": b1c_,
                "b2c": b2c_,
            }
        )
    return in_maps


# full-height output rows: partition 32q+r = pair q row r,
# partition 32q+4+r = pair 4+q row r (r = 2*two + o)
_SEL = np.array(
    [
        32 * p + r if p < 4 else 32 * (p - 4) + 4 + r
        for p in range(NPAIR)
        for r in range(4)
    ]
)


def _unshard_outputs(results):
    out = np.empty((B, D, O), np.float32)
    for c in range(NCORES):
        ot = results[c]["otB"]  # (NCH, 128, CW) fp16
        ott = ot.transpose(1, 0, 2).reshape(128, B)[_SEL].astype(np.float32)
        blk = ott.reshape(NPAIR, 2, O, B).transpose(3, 0, 1, 2).reshape(B, TPC, O)
        out[:, c * TPC : (c + 1) * TPC, :] = blk
    return out


def kernel(x, w0, w1, w2, b0, b1, b2):
    nc = _get_program()
    in_maps = _shard_inputs(x, w0, w1, w2, b0, b1, b2)
    res = bass_utils.run_bass_kernel_spmd(nc, in_maps, core_ids=list(range(NCORES)))
    return _unshard_outputs(res.results)


# revision 38
# speedup vs baseline: 1.0183x; 1.0088x over previous
"""Trainium2 Bass kernel for per-variable-MLP GNN message passing.

Model (reference):
    adj  = ones(D,D) - eye(D)                       # var t cannot see itself
    h0   = leaky_relu(einsum('tij,bj->bti', w0*adjmask, x) + b0)
    h1   = leaky_relu(einsum('tij,btj->bti', w1, h0) + b1)
    out  = einsum('tij,btj->bti', w2, h1) + b2      # (B, D, O)

Sharding: the variable axis t (128) is split across 8 cores (16 vars each);
each core sees the full batch. Within a core, variables are processed in
pairs: a pair's two (64 x K) weight matrices are stacked/block-diagonalized
to fill the 128-wide tensor-engine array; activations live transposed
(feature-on-partition, batch-on-free).

Throughput structure (v2 -- wide-tile rework; 108us -> ~95.5us):
- The PSUM->SBUF bias+leaky epilogues are the hard floor (fp32 PSUM reads
  run at 1 elem/lane/cycle and only ScalarE + VectorE have PSUM ports;
  GpSimd has none, DMA cannot touch PSUM). ScalarE runs fused
  Prelu-with-bias; VectorE runs a CUSTOM DVE op LEAKY_BIAS_ANT =
  max(z+b, alpha*(z+b)) registered at import time, so BOTH engines
  retire a tile in ONE instruction. Floor: 17 wide tiles/iteration
  ~= 9.6us/iter x 8 iters ~= 77us; both engines measure ~96% packed.
- 1024-col PSUM tiles spanning TWO banks: each epilogue instruction
  reads 1024 elems/lane, amortizing the ~175ns fixed ACT/DVE overhead
  (PSUM access latency + decode) and halving semaphore traffic on the
  two port-bound engines (ACT ~1060ns, DVE ~1210ns full-clock/tile).
  Single PSUM pool: 4 slots x [128,1024] = all 8 banks, shared by
  L0/L1/L2 in allocation order (slot WAR gives 4-alloc slack).
  MEASURED DEAD ENDS: 3 wide slots + dedicated z2 banks (+15us, the
  3-slot L0/L1 rotation stalls); interleaving the L2 block into A/B
  emission (+1..5us, early z2 alloc tightens every later slot WAR);
  chunk 0 as 2x512 sub-passes (+7us normalized); splitting every L2
  evac into 2x512 across both engines (+1.5us: the 16 extra instr
  overheads outweigh the head-of-line-blocking gap they remove);
  trimming the final DMA to valid rows / second queue (noise-level).
- Emission order per iteration k: A(L0 chunk k), B(L1 chunk k-1),
  C(L2 chunk k-2) -- C LAST is load-bearing: its first MM (q
  descending) reads the LAST-retired h1 tile, and the in-order PE
  queue stalls ~2us/iter if C is emitted before a full iteration of
  other work exists between B(k-2) and C(k-2).
- Each pair's two 512-col matmuls (same stationary weights) issue
  back-to-back at the 215ns streaming cadence; batch chunk = 1024.
- L2 packs ALL 8 pairs into one PSUM tile per 512-half at
  tile_position (0,32q) (M=8 zero-padded pB first then M=4 pA
  accumulate) -- adjacent col-group matmuls stream ~3ns apart. Last
  chunk drains q-ASCENDING with B-pairs emitted 4,5,6,7 first and the
  final evac split across both engines + two half DMAs.
- Epilogues greedily time-balanced across ScalarE/VectorE (1069:1213
  weights); h tiles are tagged per engine (ha/hv) so slot WAW deps
  stay same-engine. The L2 bias-add is Prelu(alpha=1.0) == identity
  on ScalarE / tensor_scalar_add on VectorE.
- Head (measured): ~7.2us framework boot + HAM PE clock ramp (K=4
  half-clock for ~3.4us of CONTINUOUS PE activity -- idle gaps reset
  it, hence NWARM=7 warmup MMs bridging until the input DMAs land).
  Early DMAs round-robin queue-1 bandwidth across ALL queued
  transfers, so ONLY chunk-0/1 x halves + w0 prefetch up front
  (contiguous half-chunk-major layout); later chunks + w1 stream
  just-in-time one chunk ahead from inside the loop. Tiny bias loads
  trigger from the GpSimd queue (DMA triggers cost ~607ns each and
  serialize per queue). A dummy activation hoists the 1.3us
  ACT_TABLE_LOAD into the boot window.
- Tail: ~1us drain + ~9us framework teardown (fixed semaphore-reset
  sweep + cross-queue barriers; not kernel-controllable).
- Output: full (128, 1024) fp16 tiles per chunk (one DMA each; invalid
  rows are garbage the host drops) -- host reassembles (8192, 128, 2).

Matmuls run in fp16 (1 col/cycle on the PE, fp32 accumulate in PSUM).
fp8 was evaluated and rejected: e4m3 injects ~5% rms error per layer
vs the 2e-2 tolerance (fp16 measures 6.7e-4).

NOTE run-to-run variance: the part oscillates between sustained clock
states (~18% spread, wide-ACTIVATE avg 1120ns fast vs 1330ns slow) --
normalize A/B timings by that clock proxy before trusting a comparison.
"""

import numpy as np

import concourse.bass as bass
import concourse.mybir as mybir
import concourse.tile as tile
from concourse import bacc, bass_utils


def _register_leaky_bias():
    """Register a fused leaky_relu(z + bias) custom DVE op so VectorE can
    retire a PSUM epilogue tile in ONE instruction (the PSUM read is the
    port-bound cost; stock ops need add + stt = two passes).

    out = max(in0 + s0, (in0 + s0) * s1); s0 = per-partition bias AP,
    s1 = leaky slope immediate.
    """
    from concourse import dve_ops
    from concourse.dve_spec import Spec, Src0, C0, C1, maxx, lower, _has_src1
    from concourse.dve_table_gen import dve_ver_for
    from concourse.dve_uop import DveOpSpec

    name = "LEAKY_BIAS_ANT"
    if name in dve_ops._SUB_OPCODE_FOR_NAME:
        return next(op for op in dve_ops.OPS if op.name == name)

    v = Src0 + C0
    spec = Spec(
        body=maxx(v, v * C1),
        reference=lambda in0, in1, s0, s1, imm2: np.maximum(
            in0 + s0, (in0 + s0) * s1
        ),
    )
    row = 1 + len(dve_ops.OPS)
    shas = {}
    for ver in ("v3", "v4"):
        uops = lower(spec, ver=ver)
        shas[ver] = DveOpSpec(
            name=name, opcode=row, uops=uops, rd1_en=_has_src1(spec)
        ).sha(ver)
    op = dve_ops.DveOp(name, spec, subdim=False, uops_sha=shas)
    dve_ops.OPS.append(op)
    dve_ops.CUSTOM_DVE_SPECS[name] = spec
    dve_ops._SUB_OPCODE_FOR_NAME[name] = row
    return op


LEAKY_BIAS = _register_leaky_bias()

F32 = mybir.dt.float32
DT = mybir.dt.float16
NPDT = np.float16

B = 8192  # batch
D = 128  # num variables (t)
H = 64  # hidden
O = 2  # output dim per variable
NCORES = 8
TPC = D // NCORES  # vars per core = 16
NPAIR = TPC // 2  # 8
CH = 512  # matmul moving-dim max / psum bank (fp32)
CW = 1024  # wide batch chunk = 2 psum banks per tile
NCH = B // CW  # 8
ALPHA = 0.01  # leaky_relu slope
NWARM = 7  # PE warmup matmuls (HAM clock ramp over the DMA window)
SPLIT_CHUNK0 = False  # chunk 0 as two 512 sub-passes (early-DMA latency hiding)

Prelu = mybir.ActivationFunctionType.Prelu

# per-wide-tile epilogue costs (ns, cost model): used for greedy balance
COST_ACT = 1069.0
COST_DVE = 1213.0


def _build_program():
    nc = bacc.Bacc(trn_type="TRN2")

    # x half-chunk-major: xt[h] is a contiguous (128, CH) block in DRAM
    xt = nc.dram_tensor("xt", (2 * NCH, D, CH), DT, kind="ExternalInput")
    w0t = nc.dram_tensor("w0t", (D, NPAIR * 128), DT, kind="ExternalInput")
    w1bd = nc.dram_tensor("w1bd", (128, NPAIR * 128), DT, kind="ExternalInput")
    w2bd = nc.dram_tensor("w2bd", (128, NPAIR * 8), DT, kind="ExternalInput")
    b0c = nc.dram_tensor("b0c", (128, NPAIR), F32, kind="ExternalInput")
    b1c = nc.dram_tensor("b1c", (128, NPAIR), F32, kind="ExternalInput")
    # b2c: per-partition bias; partition 32q+r = pair q row r,
    # partition 32q+4+r = pair 4+q row r
    b2c = nc.dram_tensor("b2c", (128, 1), F32, kind="ExternalInput")
    # full-height output tiles, chunk-major; host drops the garbage rows
    otB = nc.dram_tensor("otB", (NCH, 128, CW), DT, kind="ExternalOutput")

    with tile.TileContext(nc) as tc:
        with (
            tc.tile_pool(name="wp", bufs=1) as wp,
            tc.tile_pool(name="hp", bufs=36) as hp,
            tc.tile_pool(name="op", bufs=3) as op,
            tc.tile_pool(name="zp", bufs=4, space="PSUM") as zp,
        ):
            xs = wp.tile([D, B], DT)
            w0s = wp.tile([D, NPAIR * 128], DT)
            w1s = wp.tile([128, NPAIR * 128], DT)
            w2s = wp.tile([128, NPAIR * 8], DT)
            b0s = wp.tile([128, NPAIR], F32)
            b1s = wp.tile([128, NPAIR], F32)
            b2s = wp.tile([128, 1], F32)
            # trigger order = need order: the first L0 matmul is gated on
            # x half 0 + w0 pair 0 only (w0 is split so pair 0 lands fast).
            # The tiny bias/w2 loads trigger from the otherwise-idle GpSimd
            # queue: DMA triggers cost ~607ns EACH and serialize per queue,
            # so on Sync alone the b0 bias (which gates the FIRST epilogue)
            # only landed at ~12us.
            # CRITICAL-ONLY prefetch: the input DMA queue round-robins
            # bandwidth across ALL queued transfers, so prefetching every
            # x chunk up front starved the chunk-0/1 loads the pipeline
            # start gates on (they completed at ~14-16us instead of ~10).
            # Later chunks + w1 stream just-in-time from inside the loop.
            nc.sync.dma_start(xs[:, 0:CH], xt[0])
            nc.sync.dma_start(w0s[:, 0:128], w0t[:, 0:128])
            nc.sync.dma_start(w0s[:, 128:], w0t[:, 128:])
            nc.sync.dma_start(xs[:, CH:CW], xt[1])
            nc.gpsimd.dma_start(b0s[:], b0c[:])
            nc.gpsimd.dma_start(b1s[:], b1c[:])
            nc.gpsimd.dma_start(w2s[:], w2bd[:])
            nc.gpsimd.dma_start(b2s[:], b2c[:])

            # greedy ScalarE/VectorE balance over accumulated epilogue ns
            eng_t = {"act": 0.0, "cdve": 0.0}

            def pick_engine():
                k = "act" if eng_t["act"] + COST_ACT <= eng_t["cdve"] + COST_DVE else "cdve"
                eng_t[k] += COST_ACT if k == "act" else COST_DVE
                return k

            def leaky_epilogue(z, bias_col, name):
                """fp16 SBUF tile = leaky_relu(z + bias), z in PSUM.

                h tiles are segregated per engine (tags ha/hv) so the
                slot WAW dep is same-engine and folds into queue order
                instead of spilling a standalone semaphore wait."""
                kind = pick_engine()
                if kind == "act":
                    dst = hp.tile([128, CW], DT, tag="ha", bufs=20,
                                  name=name)
                    nc.scalar.activation(
                        dst[:], z[:], Prelu, bias=bias_col, scale=1.0,
                        alpha=ALPHA,
                    )
                else:  # fused leaky_relu(z + bias) in one VectorE op
                    dst = hp.tile([128, CW], DT, tag="hv", bufs=18,
                                  name=name)
                    nc.vector._custom_dve(
                        LEAKY_BIAS, out=dst[:], in0=z[:], s0=bias_col,
                        s1=ALPHA,
                    )
                return dst

            # PE warmup: dummy matmuls with no input-DMA dependency so the
            # HAM clock-gate reaches 8/8 while the input DMAs run.
            # memset on Vector (idle until ~13.5us): GpSimd's queue is
            # occupied by the bias DMA triggers above, and the warmup
            # matmuls gate the PE clock ramp -- warm must land ASAP
            warm = wp.tile([128, CH], DT, name="warm")
            nc.vector.memset(warm[:], 0.0)
            # dummy activation: hoists the 1.3us ACT_TABLE_LOAD (emitted
            # lazily before the first ACTIVATE) into the DMA window -- it
            # has no input deps but was measured running at ~12us behind
            # the scheduler's hoisted DMA wait
            dumm = wp.tile([128, 1], DT, name="dumm")
            nc.scalar.activation(dumm[:], warm[:, 0:1], Prelu, scale=1.0,
                                 alpha=ALPHA)
            wps = zp.tile([128, CW], F32, name="warmps", tag="z")
            for _ in range(NWARM):
                nc.tensor.matmul(wps[:, 0:CH], warm[:, 0:128], warm[:],
                                 start=True, stop=True)

            # per-chunk state carried across pipeline stages
            h0_tiles = [None] * NCH
            h1_tiles = [None] * NCH

            def emit_A_pair(k, p):
                z0 = zp.tile([128, CW], F32, tag="z", name=f"z0_{k}_{p}")
                for half in range(2):
                    nc.tensor.matmul(
                        z0[:, half * CH : half * CH + CH],
                        w0s[:, bass.ts(p, 128)],
                        xs[:, k * CW + half * CH : k * CW + half * CH + CH],
                        start=True, stop=True,
                    )
                return leaky_epilogue(z0, b0s[:, p : p + 1], f"h0_{k}_{p}")

            def emit_A_chunk0():
                """Chunk 0 runs as two 512-wide sub-passes: the early input
                DMAs have ~2-3us latency, so the first wide epilogue would
                otherwise idle both port engines until x half 1 lands
                (~12us); half-0-only work starts ~2.5us sooner."""
                tiles = []
                for p in range(NPAIR):
                    tag = "ha" if p % 2 == 0 else "hv"
                    bufs = 20 if tag == "ha" else 18
                    tiles.append(hp.tile([128, CW], DT, tag=tag, bufs=bufs,
                                         name=f"h0_0_{p}"))
                for half in range(2):
                    for p in range(NPAIR):
                        z0 = zp.tile([128, CW], F32, tag="z",
                                     name=f"z0_0_{p}_{half}")
                        nc.tensor.matmul(
                            z0[:, 0:CH], w0s[:, bass.ts(p, 128)],
                            xs[:, half * CH : half * CH + CH],
                            start=True, stop=True,
                        )
                        dst = tiles[p][:, half * CH : half * CH + CH]
                        if pick_engine() == "act":
                            nc.scalar.activation(
                                dst, z0[:, 0:CH], Prelu,
                                bias=b0s[:, p : p + 1], scale=1.0,
                                alpha=ALPHA,
                            )
                        else:
                            nc.vector._custom_dve(
                                LEAKY_BIAS, out=dst, in0=z0[:, 0:CH],
                                s0=b0s[:, p : p + 1], s1=ALPHA,
                            )
                return tiles

            def emit_B_pair(c, p):
                z1 = zp.tile([128, CW], F32, tag="z", name=f"z1_{c}_{p}")
                t0 = h0_tiles[c][p]
                for half in range(2):
                    nc.tensor.matmul(
                        z1[:, half * CH : half * CH + CH],
                        w1s[:, bass.ts(p, 128)],
                        t0[:, half * CH : half * CH + CH],
                        start=True, stop=True,
                    )
                return leaky_epilogue(z1, b1s[:, p : p + 1], f"h1_{c}_{p}")

            def emit_C_half(c, z2, half, qorder):
                hs = slice(half * CH, half * CH + CH)
                for q in qorder:
                    # pair 4+q: M=8 zero-padded weights write zeros into
                    # rows 32q..+4, data into rows 32q+4..+8
                    pB = 4 + q
                    nc.tensor.matmul(
                        z2[32 * q : 32 * q + 8, hs],
                        w2s[:, 8 * pB : 8 * pB + 8],
                        h1_tiles[c][pB][:, hs],
                        start=True, stop=False,
                        tile_position=(0, 32 * q),
                    )
                for q in qorder:
                    # pair q accumulates into rows 32q..32q+4
                    nc.tensor.matmul(
                        z2[32 * q : 32 * q + 4, hs],
                        w2s[:, 8 * q : 8 * q + 4],
                        h1_tiles[c][q][:, hs],
                        start=False, stop=True,
                        tile_position=(0, 32 * q),
                    )

            def emit_C_evac(c, z2):
                if c == NCH - 1:
                    # drain fast: one 512 half per engine, two DMAs
                    ob = op.tile([128, CW], DT, tag="oba")
                    nc.scalar.activation(
                        ob[:, 0:CH], z2[:, 0:CH], Prelu,
                        bias=b2s[:, 0:1], scale=1.0, alpha=1.0,
                    )
                    nc.vector.tensor_scalar_add(
                        ob[:, CH:CW], z2[:, CH:CW], b2s[:, 0:1]
                    )
                    nc.sync.dma_start(otB[c][:, 0:CH], ob[:, 0:CH])
                    nc.sync.dma_start(otB[c][:, CH:CW], ob[:, CH:CW])
                elif pick_engine() == "act":
                    ob = op.tile([128, CW], DT, tag="oba")
                    # Prelu with alpha=1 is identity: bias-add on ACT
                    nc.scalar.activation(
                        ob[:], z2[:], Prelu, bias=b2s[:, 0:1],
                        scale=1.0, alpha=1.0,
                    )
                    nc.sync.dma_start(otB[c], ob[:])
                else:
                    ob = op.tile([128, CW], DT, tag="obv")
                    nc.vector.tensor_scalar_add(ob[:], z2[:], b2s[:, 0:1])
                    nc.sync.dma_start(otB[c], ob[:])
                h1_tiles[c] = None

            for k in range(NCH + 2):
                cA = k if k < NCH else None
                cB = k - 1 if 1 <= k <= NCH else None
                cC = k - 2 if k >= 2 else None
                # last chunk: emit the pB pairs (4-7) first and run the L2
                # drain q-ASCENDING so its first matmuls read the
                # FIRST-retired h1 tiles and overlap the tail epilogues
                qorder = range(4) if cC == NCH - 1 else range(3, -1, -1)
                pordB = (
                    [4, 5, 6, 7, 0, 1, 2, 3] if cB == NCH - 1
                    else list(range(NPAIR))
                )

                # main-stage emission units to interleave the L2 rounds
                # with: the 16-MM L2 block alone occupies ~1us of the
                # in-order PE queue while producing one epilogue tile, so
                # emitted standalone it runs both port engines dry at every
                # iteration boundary (~0.5us each, measured)
                units = []
                if cA is not None:
                    units += [("A", cA, p) for p in range(NPAIR)]
                if cB is not None:
                    units += [("B", cB, p) for p in pordB]

                outA = {}
                outB = {}
                # NOTE: interleaving the L2 block into the A/B emission was
                # measured SLOWER (+2..5us): the early z2 slot allocation
                # tightens the 4-slot WAR window for every subsequent tile.
                # Emitting C after all of A and B (z2 = last alloc of the
                # iteration) is the fastest measured arrangement.
                # just-in-time input streaming, one chunk ahead (chunk
                # k+1 loads during iteration k, consumed in iteration k+1)
                if k == 0:
                    nc.sync.dma_start(w1s[:], w1bd[:])
                if k + 1 < NCH:
                    for hh in (2 * (k + 1), 2 * (k + 1) + 1):
                        nc.sync.dma_start(
                            xs[:, hh * CH : (hh + 1) * CH], xt[hh]
                        )

                if k == 0 and SPLIT_CHUNK0:
                    h0_tiles[0] = emit_A_chunk0()
                    units = [u for u in units if u[0] != "A"]
                for st, c, p in units:
                    if st == "A":
                        outA[p] = emit_A_pair(c, p)
                    else:
                        outB[p] = emit_B_pair(c, p)
                if cC is not None:
                    z2 = zp.tile([128, CW], F32, name=f"z2_{cC}", tag="z")
                    emit_C_half(cC, z2, 0, qorder)
                    emit_C_half(cC, z2, 1, qorder)
                    emit_C_evac(cC, z2)
                if cA is not None and not (cA == 0 and SPLIT_CHUNK0):
                    h0_tiles[cA] = [outA[p] for p in range(NPAIR)]
                if cB is not None:
                    h1_tiles[cB] = [outB[p] for p in range(NPAIR)]
                    h0_tiles[cB] = None

    nc.finalize()
    return nc


_prog = None


def _get_program():
    global _prog
    if _prog is None:
        _prog = _build_program()
    return _prog


def _shard_inputs(x, w0, w1, w2, b0, b1, b2):
    """Host-side relayout + t-sharding. Returns list of 8 in_maps."""
    x = np.asarray(x, np.float32)
    w0 = np.array(w0, np.float32)  # copy: we zero the adjacency diagonal
    w1 = np.asarray(w1, np.float32)
    w2 = np.asarray(w2, np.float32)
    b0 = np.asarray(b0, np.float32)
    b1 = np.asarray(b1, np.float32)
    b2 = np.asarray(b2, np.float32)

    # adjacency mask: variable t cannot see itself -> w0[t, :, t] = 0
    ar = np.arange(D)
    w0[ar, :, ar] = 0.0

    # half-chunk-major x: (2*NCH, 128, CH), each half contiguous in DRAM
    xt = np.ascontiguousarray(
        x.T.reshape(D, 2 * NCH, CH).transpose(1, 0, 2)
    ).astype(NPDT)

    in_maps = []
    for c in range(NCORES):
        ts_ = slice(c * TPC, (c + 1) * TPC)
        w0c, w1c, w2c = w0[ts_], w1[ts_], w2[ts_]
        b0cc, b1cc, b2cc = b0[ts_], b1[ts_], b2[ts_]

        # w0t: (128 j, pair*128 + [ta's 64 i | tb's 64 i])
        w0T = w0c.transpose(0, 2, 1)  # (16, 128 j, 64 i)
        w0t_ = np.ascontiguousarray(
            w0T.reshape(NPAIR, 2, D, H).transpose(2, 0, 1, 3).reshape(D, NPAIR * 128)
        ).astype(NPDT)

        # w1bd: per-pair 128x128 block-diagonal; K rows = h0 pair stack.
        bd1 = np.zeros((NPAIR, 128, 128), np.float32)
        for p in range(NPAIR):
            te, to = w1c[2 * p].T, w1c[2 * p + 1].T  # (in, out) each (64,64)
            bd1[p, 0:H, 0:H] = te
            bd1[p, H:128, H:128] = to
        w1bd_ = np.ascontiguousarray(
            bd1.transpose(1, 0, 2).reshape(128, NPAIR * 128)
        ).astype(NPDT)

        b1c_ = np.ascontiguousarray(
            b1cc.reshape(NPAIR, 128).T
        ).astype(np.float32)

        # w2bd: (128 K, pair*8 + [4 zero-pad | ta o0, ta o1, tb o0, tb o1]);
        # B pairs sit 4 cols right (zero-padded M=8 write clears the rows
        # the A pairs later accumulate into)
        bd2 = np.zeros((NPAIR, 128, 8), np.float32)
        for p in range(NPAIR):
            te, to = w2c[2 * p].T, w2c[2 * p + 1].T  # (64, 2) each
            off = 0 if p < 4 else 4
            bd2[p, 0:H, off : off + 2] = te
            bd2[p, H:128, off + 2 : off + 4] = to
        w2bd_ = np.ascontiguousarray(
            bd2.transpose(1, 0, 2).reshape(128, NPAIR * 8)
        ).astype(NPDT)

        b0c_ = np.ascontiguousarray(b0cc.reshape(NPAIR, 128).T).astype(np.float32)
        # b2 bias layout for col-packed L2 psums: partition 32q+r = pair q
        # row r; partition 32q+4+r = pair 4+q row r (r = 2*two + o)
        b2q = b2cc.reshape(NPAIR, 4)
        b2c_ = np.zeros((128, 1), np.float32)
        for q in range(4):
            b2c_[32 * q : 32 * q + 4, 0] = b2q[q]
            b2c_[32 * q + 4 : 32 * q + 8, 0] = b2q[4 + q]

        in_maps.append(
            {
                "xt": xt,
                "w0t": w0t_,
                "w1bd": w1bd_,
                "w2bd": w2bd_,
                "b0c": b0c_,
                "b1c": b1c_,
                "b2c": b2c_,
            }
        )
    return in_maps


# full-height output rows: partition 32q+r = pair q row r,
# partition 32q+4+r = pair 4+q row r (r = 2*two + o)
_SEL = np.array(
    [
        32 * p + r if p < 4 else 32 * (p - 4) + 4 + r
        for p in range(NPAIR)
        for r in range(4)
    ]
)


def _unshard_outputs(results):
    out = np.empty((B, D, O), np.float32)
    for c in range(NCORES):
        ot = results[c]["otB"]  # (NCH, 128, CW) fp16
        ott = ot.transpose(1, 0, 2).reshape(128, B)[_SEL].astype(np.float32)
        blk = ott.reshape(NPAIR, 2, O, B).transpose(3, 0, 1, 2).reshape(B, TPC, O)
        out[:, c * TPC : (c + 1) * TPC, :] = blk
    return out


def kernel(x, w0, w1, w2, b0, b1, b2):
    nc = _get_program()
    in_maps = _shard_inputs(x, w0, w1, w2, b0, b1, b2)
    res = bass_utils.run_bass_kernel_spmd(nc, in_maps, core_ids=list(range(NCORES)))
    return _unshard_outputs(res.results)
